# revision 16
# baseline (speedup 1.0000x reference)
"""GCNModelVAE on 8 Trainium2 NeuronCores (Bass/Tile SPMD kernel).

Computes:
    h1     = S @ (x @ W1) + b1          S = D^-1/2 (A + I) D^-1/2
    mu     = S @ (h1 @ W2) + b2
    logvar = S @ (h1 @ W3) + b3
    adj    = sigmoid(mu @ mu.T)

Sharding: 1-D row partition of nodes across 8 cores (2048 nodes each).

Message passing: per-node padded source-slot tables.  The degree-prescaled
feature tables are all-gathered in fp16 (one 256-byte row per node, plus one
zero stripe per rank for padding slots), loaded into SBUF, and gathered with
the SBUF-source transpose-mode dma_gather: output is feature-major
[128 feat, n_idxs], so the per-node slot sum is a contiguous-inner DVE
reduce and directly yields transposed features (h1T / muT) for the next
matmul.  deg is computed on-device by counting pad slots.
"""

import numpy as np

N_CORES = 8

# ---------------------------------------------------------------------------
# Device kernel builder
# ---------------------------------------------------------------------------


def build_kernel(n_nodes, nsh, s_slots, f_in, h1, h2, phases=4):
    """Build + compile the SPMD Bass program.  Returns the Bacc object."""
    import concourse.bacc as bacc
    import concourse.mybir as mybir
    import concourse.tile as tile
    from concourse.masks import make_identity

    fp = mybir.dt.float32
    f16 = mybir.dt.float16
    i16 = mybir.dt.int16
    n_blk = nsh // 128
    wcols = (128 * s_slots) // 16  # wrapped idx columns per block
    GIDX = 512  # idxs per dma_gather call
    h23 = 2 * h2
    AF = mybir.ActivationFunctionType
    OP = mybir.AluOpType
    ST = n_blk + 1  # table stripes per rank (n_blk node stripes + 1 zero)
    TROW = 128  # fp16 elements per table row (256 bytes)

    nc = bacc.Bacc(
        "TRN2",
        target_bir_lowering=False,
        debug=False,
        enable_asserts=False,
        num_devices=N_CORES,
    )

    xs = nc.dram_tensor("xs", [nsh, f_in], fp, kind="ExternalInput").ap()
    w1 = nc.dram_tensor("w1", [f_in, h1], fp, kind="ExternalInput").ap()
    w23 = nc.dram_tensor("w23", [h1, h23], fp, kind="ExternalInput").ap()
    b1c = nc.dram_tensor("b1c", [128, 1], fp, kind="ExternalInput").ap()
    b23c = nc.dram_tensor("b23c", [128, 1], fp, kind="ExternalInput").ap()
    idxw = nc.dram_tensor("idxw", [128, n_blk * wcols], i16, kind="ExternalInput").ap()
    idxp = nc.dram_tensor("idxp", [nsh, s_slots], fp, kind="ExternalInput").ap()

    adj = nc.dram_tensor("adj", [nsh, n_nodes], fp, kind="ExternalOutput").ap()
    muo = nc.dram_tensor("muo", [nsh, h2], fp, kind="ExternalOutput").ap()
    lvo = nc.dram_tensor("lvo", [nsh, h2], fp, kind="ExternalOutput").ap()

    rg = [list(range(N_CORES))]

    with tile.TileContext(nc) as tc:
        with (
            tc.tile_pool(name="sb", bufs=1) as sb,
            tc.tile_pool(name="wk", bufs=2) as wk,
            tc.tile_pool(name="ps", bufs=2, space="PSUM") as ps,
            tc.tile_pool(name="dram", bufs=1, space="DRAM") as dr,
        ):
            # ---------------- persistent setup ----------------
            ident = sb.tile([128, 128], fp)
            make_identity(nc, ident[:])

            w1sb = sb.tile([128, 4 * h1], fp)  # W1 as 4 K-chunks side by side
            for k in range(f_in // 128):
                nc.sync.dma_start(
                    w1sb[:, k * h1 : (k + 1) * h1], w1[k * 128 : (k + 1) * 128, :]
                )
            w23sb = sb.tile([h1, h23], fp)
            nc.sync.dma_start(w23sb[:], w23[:])
            b1sb = sb.tile([128, 1], fp)
            nc.sync.dma_start(b1sb[:], b1c[:])
            b23sb = sb.tile([128, 1], fp)
            nc.sync.dma_start(b23sb[:], b23c[:])
            ones1 = sb.tile([1, 128], fp)
            nc.vector.memset(ones1[:], 1.0)

            # gather index table (int16, wrapped layout), resident in SBUF
            idx_sb = sb.tile([128, n_blk * wcols], i16)
            nc.sync.dma_start(idx_sb[:], idxw[:])

            dinv = sb.tile([128, n_blk], fp)  # per-block node-major columns
            dinv128 = sb.tile([128, nsh], fp)  # broadcast across partitions
            h1T = sb.tile([128, nsh], fp)  # rows 0..h1-1 used
            muLT = sb.tile([128, nsh], fp)  # rows 0..h23-1 = [muT; logvarT]
            cc1_sb = sb.tile([128, ST * TROW], f16)
            cc2_sb = sb.tile([128, ST * TROW], f16)
            tbl = sb.tile([128, N_CORES * ST * TROW], f16)
            # zT split into 4 column-group tiles (all at partition base 0)
            cpg = n_nodes // 4  # columns per group
            ztg = [sb.tile([h2, cpg], fp, name=f"ztg{q}") for q in range(4)]

            nc.vector.memset(cc1_sb[:], 0.0)
            nc.vector.memset(cc2_sb[:], 0.0)

            # internal DRAM
            cc1_in = dr.tile([128, ST * TROW], f16)
            t1 = dr.tile([N_CORES * 128, ST * TROW], f16, addr_space="Shared")
            cc2_in = dr.tile([128, ST * TROW], f16)
            t2 = dr.tile([N_CORES * 128, ST * TROW], f16, addr_space="Shared")
            cc3_in = dr.tile([h2, nsh], fp)
            ztd = dr.tile([h2 * N_CORES, nsh], fp, addr_space="Shared")

            # ---------------- phase A: xw = x @ W1, deg, dinv, table 1 -------
            for b in range(n_blk):
                r0 = b * 128
                xb = wk.tile([128, f_in], fp, name="xb")
                nc.sync.dma_start(xb[:], xs[r0 : r0 + 128, :])
                xt4 = wk.tile([128, 4 * 128], fp, name="xt4")
                for k in range(f_in // 128):
                    tp = ps.tile([128, 128], fp, name="tp", tag="tp")
                    nc.tensor.transpose(
                        tp[:], xb[:, k * 128 : (k + 1) * 128], ident[:]
                    )
                    nc.vector.tensor_copy(xt4[:, k * 128 : (k + 1) * 128], tp[:])
                xwp = ps.tile([128, h1], fp, name="xwp", tag="acc")
                for k in range(f_in // 128):
                    nc.tensor.matmul(
                        xwp[:],
                        lhsT=xt4[:, k * 128 : (k + 1) * 128],
                        rhs=w1sb[:, k * h1 : (k + 1) * h1],
                        start=(k == 0),
                        stop=(k == f_in // 128 - 1),
                    )

                # degree from pad counts: deg = s_slots - #(idx == n_nodes)
                ixp = wk.tile([128, s_slots], fp, name="ixp")
                nc.sync.dma_start(ixp[:], idxp[r0 : r0 + 128, :])
                eq = wk.tile([128, s_slots], fp, name="eq")
                nc.vector.tensor_scalar(
                    eq[:], ixp[:], float(n_nodes), None, op0=OP.is_equal
                )
                cnt = wk.tile([128, 1], fp, name="cnt")
                nc.vector.reduce_sum(cnt[:], eq[:], axis=mybir.AxisListType.X)
                degt = wk.tile([128, 1], fp, name="degt")
                nc.vector.tensor_scalar(
                    degt[:], cnt[:], -1.0, float(s_slots), op0=OP.mult, op1=OP.add
                )
                rec = wk.tile([128, 1], fp, name="rec")
                nc.vector.reciprocal(rec[:], degt[:])
                nc.scalar.sqrt(dinv[:, b : b + 1], rec[:])

                # table row: dinv-scaled xw, fp16, features in cols 0..h1
                nc.vector.tensor_scalar(
                    cc1_sb[:, b * TROW : b * TROW + h1],
                    xwp[:],
                    dinv[:, b : b + 1],
                    None,
                    op0=OP.mult,
                )

            # dinv broadcast across partitions: per block transpose the
            # free-broadcast column (out[j, p] = dinv[p, b] for all j)
            for b in range(n_blk):
                dbp = ps.tile([128, 128], fp, name="dbp", tag="tp")
                nc.tensor.transpose(
                    dbp[:], dinv[:, b : b + 1].to_broadcast([128, 128]), ident[:]
                )
                nc.vector.tensor_copy(dinv128[:, b * 128 : (b + 1) * 128], dbp[:])

            nc.sync.dma_start(cc1_in[:], cc1_sb[:])
            nc.gpsimd.collective_compute(
                "AllGather",
                mybir.AluOpType.bypass,
                replica_groups=rg,
                ins=[cc1_in[:]],
                outs=[t1[:]],
            )
            # load gathered table into SBUF: partition p <- t1 row r*128+p
            nc.sync.dma_start(
                tbl[:].rearrange("p (r e) -> p r e", r=N_CORES),
                t1[:].rearrange("(r p) e -> p r e", p=128),
            )

            def gather_reduce(b):
                """Gather this block's slots from `tbl`, slot-sum -> [128,128] f16."""
                g1 = wk.tile([128, 128 * s_slots], f16, name="g1")
                ncall = (128 * s_slots + GIDX - 1) // GIDX
                for gci in range(ncall):
                    i0 = gci * GIDX
                    i1 = min(i0 + GIDX, 128 * s_slots)
                    nc.gpsimd.dma_gather(
                        out_ap=g1[:, i0:i1].rearrange("p (a c) -> p a c", a=1),
                        in_ap=tbl[:],
                        idxs_ap=idx_sb[
                            :, b * wcols + i0 // 16 : b * wcols + i1 // 16
                        ],
                        num_idxs=i1 - i0,
                        num_idxs_reg=i1 - i0,
                        elem_size=TROW,
                        transpose=True,
                        sbuf_tokens_per_rank=128,
                        sbuf_free_dim_per_rank=2 * TROW,
                    )
                red = wk.tile([128, 128], fp, name="red")
                nc.vector.reduce_sum(
                    red[:],
                    g1[:].rearrange("p (n s) -> p n s", s=s_slots),
                    axis=mybir.AxisListType.X,
                )
                return red

            # ---------------- phase B: h1T, table 2 ----------------
            import os
            B_PARTS = int(os.environ.get("B_PARTS", "4"))
            for b in range(n_blk if phases >= 2 else 0):
                r0 = b * 128
                red = gather_reduce(b)
                if B_PARTS < 2:
                    nc.vector.tensor_copy(h1T[:, r0 : r0 + 128], red[:])
                    continue
                # h1T = red * dinv(node, free) + b1(feature, partition)
                nc.vector.tensor_tensor(
                    h1T[:, r0 : r0 + 128],
                    red[:],
                    dinv128[:, r0 : r0 + 128],
                    op=OP.mult,
                )
                nc.vector.tensor_scalar(
                    h1T[:, r0 : r0 + 128],
                    h1T[:, r0 : r0 + 128],
                    b1sb[:, :1],
                    None,
                    op0=OP.add,
                )
                if B_PARTS < 3:
                    continue
                # hw23 = (h1 @ W23) node-major, then dinv-scale into table 2
                hwp = ps.tile([128, h23], fp, name="hwp", tag="acc")
                nc.tensor.matmul(
                    hwp[:],
                    lhsT=h1T[:h1, r0 : r0 + 128],
                    rhs=w23sb[:],
                    start=True,
                    stop=True,
                )
                nc.vector.tensor_scalar(
                    cc2_sb[:, b * TROW : b * TROW + h23],
                    hwp[:],
                    dinv[:, b : b + 1],
                    None,
                    op0=OP.mult,
                )

            if phases >= 2:
                nc.sync.dma_start(cc2_in[:], cc2_sb[:])
                nc.gpsimd.collective_compute(
                    "AllGather",
                    mybir.AluOpType.bypass,
                    replica_groups=rg,
                    ins=[cc2_in[:]],
                    outs=[t2[:]],
                )
                nc.sync.dma_start(
                    tbl[:].rearrange("p (r e) -> p r e", r=N_CORES),
                    t2[:].rearrange("(r p) e -> p r e", p=128),
                )

            # ---------------- phase C: muT / logvarT -----------------------
            for b in range(n_blk if phases >= 3 else 0):
                r0 = b * 128
                red = gather_reduce(b)
                nc.vector.tensor_tensor(
                    muLT[:, r0 : r0 + 128],
                    red[:],
                    dinv128[:, r0 : r0 + 128],
                    op=OP.mult,
                )
                nc.vector.tensor_scalar(
                    muLT[:, r0 : r0 + 128],
                    muLT[:, r0 : r0 + 128],
                    b23sb[:, :1],
                    None,
                    op0=OP.add,
                )
                # node-major [128, h23] for mu / logvar outputs
                mtp = ps.tile([128, h23], fp, name="mtp", tag="tp")
                nc.tensor.transpose(
                    mtp[:], muLT[:h23, r0 : r0 + 128], ident[:h23, :h23]
                )
                ml = wk.tile([128, h23], fp, name="ml")
                nc.vector.tensor_copy(ml[:], mtp[:])
                nc.sync.dma_start(muo[r0 : r0 + 128, :], ml[:, :h2])
                nc.sync.dma_start(lvo[r0 : r0 + 128, :], ml[:, h2:])

            if phases >= 3:
                nc.sync.dma_start(cc3_in[:], muLT[:h2, :])
                nc.gpsimd.collective_compute(
                    "AllGather",
                    mybir.AluOpType.bypass,
                    replica_groups=rg,
                    ins=[cc3_in[:]],
                    outs=[ztd[:]],
                )
                # rearrange rank stripes into the 4 column-group tiles
                for r in range(N_CORES):
                    q, s = r // 2, r % 2
                    nc.sync.dma_start(
                        ztg[q][:, s * nsh : (s + 1) * nsh],
                        ztd[h2 * r : h2 * (r + 1), :],
                    )

            # ---------------- phase D: adj = sigmoid(z z^T) -----------------
            CW = 2048 if n_nodes % 2048 == 0 else 512  # columns per output DMA
            cpg512 = cpg // 512  # 512-col chunks per group
            for rb in range(n_blk if phases >= 4 else 0):
                r0 = rb * 128
                for cg in range(n_nodes // CW):
                    adjsb = wk.tile([128, CW], fp, name="adjsb")
                    for cc in range(CW // 512):
                        c = cg * (CW // 512) + cc
                        q = c // cpg512
                        lc = c % cpg512
                        pa = ps.tile([128, 512], fp, name="pa")
                        nc.tensor.matmul(
                            pa[:],
                            lhsT=muLT[:h2, r0 : r0 + 128],
                            rhs=ztg[q][:, lc * 512 : (lc + 1) * 512],
                            start=True,
                            stop=True,
                        )
                        nc.scalar.activation(
                            adjsb[:, cc * 512 : (cc + 1) * 512], pa[:], AF.Sigmoid
                        )
                    nc.sync.dma_start(
                        adj[r0 : r0 + 128, cg * CW : (cg + 1) * CW], adjsb[:]
                    )

    nc.compile()
    return nc


# ---------------------------------------------------------------------------
# Host-side sharding / index construction
# ---------------------------------------------------------------------------


def build_inputs(x, edge_index, W1, b1, W2, b2, W3, b3, n_nodes, s_slots=None):
    """Build per-core input maps.  Returns (in_maps, s_slots)."""
    N = n_nodes
    nsh = N // N_CORES
    n_blk = nsh // 128

    src = np.concatenate(
        [np.asarray(edge_index[0]), np.arange(N, dtype=np.int64)]
    ).astype(np.int64)
    dst = np.concatenate(
        [np.asarray(edge_index[1]), np.arange(N, dtype=np.int64)]
    ).astype(np.int64)
    deg = np.bincount(dst, minlength=N)
    if s_slots is None:
        s_slots = max(64, int(deg.max()))
    assert deg.max() <= s_slots

    order = np.argsort(dst, kind="stable")
    sdst = dst[order]
    ssrc = src[order]
    offs = np.zeros(N + 1, np.int64)
    np.cumsum(deg, out=offs[1:])
    slot = np.arange(len(sdst), dtype=np.int64) - offs[sdst]
    tbl = np.full((N, s_slots), N, np.int64)
    tbl[sdst, slot] = ssrc

    wcols = (128 * s_slots) // 16
    x = np.ascontiguousarray(np.asarray(x, np.float32))
    w1 = np.ascontiguousarray(np.asarray(W1, np.float32))
    w23 = np.ascontiguousarray(
        np.concatenate([np.asarray(W2), np.asarray(W3)], axis=1).astype(np.float32)
    )
    h2 = np.asarray(W2).shape[1]
    b1cv = np.zeros((128, 1), np.float32)
    b1cv[: len(np.asarray(b1)), 0] = np.asarray(b1, np.float32)
    b23cv = np.zeros((128, 1), np.float32)
    b23cv[:h2, 0] = np.asarray(b2, np.float32)
    b23cv[h2 : 2 * h2, 0] = np.asarray(b3, np.float32)

    # token id for node g: rank r = g//nsh, stripe st = (g%nsh)//128,
    # partition p = g%128  ->  (r*(n_blk+1) + st)*128 + p.
    # pad (value N) -> rank 0's zero stripe, token n_blk*128.
    ST = n_blk + 1
    r_ = tbl // nsh
    loc = tbl % nsh
    tok = (r_ * ST + loc // 128) * 128 + (tbl % 128)
    tok = np.where(tbl == N, n_blk * 128, tok)

    in_maps = []
    for r in range(N_CORES):
        lo = r * nsh
        tok_r = tok[lo : lo + nsh]
        idxw = np.zeros((128, n_blk * wcols), np.int16)
        for b in range(n_blk):
            # node-major stream: i = n_local*s_slots + s
            unw = tok_r[b * 128 : (b + 1) * 128].reshape(-1)
            blk = unw.reshape(wcols, 16).T.astype(np.int16)
            for kq in range(8):  # replicated per gpsimd core (16-part groups)
                idxw[16 * kq : 16 * (kq + 1), b * wcols : (b + 1) * wcols] = blk
        in_maps.append(
            {
                "xs": np.ascontiguousarray(x[lo : lo + nsh]),
                "w1": w1,
                "w23": w23,
                "b1c": b1cv,
                "b23c": b23cv,
                "idxw": idxw,
                "idxp": tbl[lo : lo + nsh].astype(np.float32),
            }
        )
    return in_maps, s_slots


# ---------------------------------------------------------------------------
# Public entry point
# ---------------------------------------------------------------------------

_CACHE = {}


def run(x, edge_index, W1, b1, W2, b2, W3, b3, num_nodes, trace=False, **kw):
    from concourse.bass_utils import run_bass_kernel_spmd

    N = int(num_nodes)
    f_in = x.shape[1]
    h1 = W1.shape[1]
    h2 = W2.shape[1]
    nsh = N // N_CORES

    in_maps, s_slots = build_inputs(
        x, edge_index, W1, b1, W2, b2, W3, b3, N
    )
    key = (N, nsh, s_slots, f_in, h1, h2)
    if key not in _CACHE:
        _CACHE[key] = build_kernel(N, nsh, s_slots, f_in, h1, h2)
    nc = _CACHE[key]

    res = run_bass_kernel_spmd(
        nc, in_maps, core_ids=list(range(N_CORES)), trace=trace, **kw
    )
    outs = res.results
    adj = np.concatenate([o["adj"] for o in outs], axis=0)
    mu = np.concatenate([o["muo"] for o in outs], axis=0)
    lv = np.concatenate([o["lvo"] for o in outs], axis=0)
    return (adj, mu, lv), res


def kernel(x, edge_index, W1, b1, W2, b2, W3, b3, num_nodes):
    (adj, mu, lv), _ = run(x, edge_index, W1, b1, W2, b2, W3, b3, num_nodes)
    return adj, mu, lv


# revision 17
# speedup vs baseline: 1.0968x; 1.0968x over previous
"""GCNModelVAE on 8 Trainium2 NeuronCores (Bass/Tile SPMD kernel).

Computes:
    h1     = S @ (x @ W1) + b1          S = D^-1/2 (A + I) D^-1/2
    mu     = S @ (h1 @ W2) + b2
    logvar = S @ (h1 @ W3) + b3
    adj    = sigmoid(mu @ mu.T)

Sharding: 1-D row partition of nodes across 8 cores (2048 nodes each).

Message passing: per-node padded source-slot tables.  The degree-prescaled
feature tables are all-gathered in fp16 (one 256-byte row per node, plus one
zero stripe per rank for padding slots), loaded into SBUF, and gathered with
the SBUF-source transpose-mode dma_gather: output is feature-major
[128 feat, n_idxs], so the per-node slot sum is a contiguous-inner DVE
reduce and directly yields transposed features (h1T / muT) for the next
matmul.  deg is computed on-device by counting pad slots.
"""

import numpy as np

N_CORES = 8

# ---------------------------------------------------------------------------
# Device kernel builder
# ---------------------------------------------------------------------------


def build_kernel(n_nodes, nsh, s_slots, f_in, h1, h2, phases=4):
    """Build + compile the SPMD Bass program.  Returns the Bacc object."""
    import concourse.bacc as bacc
    import concourse.mybir as mybir
    import concourse.tile as tile
    from concourse.masks import make_identity

    fp = mybir.dt.float32
    f16 = mybir.dt.float16
    i16 = mybir.dt.int16
    n_blk = nsh // 128
    wcols = (128 * s_slots) // 16  # wrapped idx columns per block
    GIDX = 2048  # idxs per dma_gather call
    h23 = 2 * h2
    AF = mybir.ActivationFunctionType
    OP = mybir.AluOpType
    ST = n_blk + 1  # table stripes per rank (n_blk node stripes + 1 zero)
    TROW = 128  # fp16 elements per table row (256 bytes)

    nc = bacc.Bacc(
        "TRN2",
        target_bir_lowering=False,
        debug=False,
        enable_asserts=False,
        num_devices=N_CORES,
    )

    xs = nc.dram_tensor("xs", [nsh, f_in], fp, kind="ExternalInput").ap()
    w1 = nc.dram_tensor("w1", [f_in, h1], fp, kind="ExternalInput").ap()
    w23 = nc.dram_tensor("w23", [h1, h23], fp, kind="ExternalInput").ap()
    b1c = nc.dram_tensor("b1c", [128, 1], fp, kind="ExternalInput").ap()
    b23c = nc.dram_tensor("b23c", [128, 1], fp, kind="ExternalInput").ap()
    idxw = nc.dram_tensor("idxw", [128, n_blk * wcols], i16, kind="ExternalInput").ap()
    idxp = nc.dram_tensor("idxp", [nsh, s_slots], fp, kind="ExternalInput").ap()

    adj = nc.dram_tensor("adj", [nsh, n_nodes], fp, kind="ExternalOutput").ap()
    muo = nc.dram_tensor("muo", [nsh, h2], fp, kind="ExternalOutput").ap()
    lvo = nc.dram_tensor("lvo", [nsh, h2], fp, kind="ExternalOutput").ap()

    rg = [list(range(N_CORES))]

    with tile.TileContext(nc) as tc:
        with (
            tc.tile_pool(name="sb", bufs=1) as sb,
            tc.tile_pool(name="wk", bufs=2) as wk,
            tc.tile_pool(name="ps", bufs=2, space="PSUM") as ps,
            tc.tile_pool(name="dram", bufs=1, space="DRAM") as dr,
        ):
            # ---------------- persistent setup ----------------
            ident = sb.tile([128, 128], fp)
            make_identity(nc, ident[:])

            w1sb = sb.tile([128, 4 * h1], fp)  # W1 as 4 K-chunks side by side
            for k in range(f_in // 128):
                nc.sync.dma_start(
                    w1sb[:, k * h1 : (k + 1) * h1], w1[k * 128 : (k + 1) * 128, :]
                )
            w23sb = sb.tile([h1, h23], fp)
            nc.sync.dma_start(w23sb[:], w23[:])
            b1sb = sb.tile([128, 1], fp)
            nc.sync.dma_start(b1sb[:], b1c[:])
            b23sb = sb.tile([128, 1], fp)
            nc.sync.dma_start(b23sb[:], b23c[:])
            ones1 = sb.tile([1, 128], fp)
            nc.vector.memset(ones1[:], 1.0)

            # gather index table (int16, wrapped layout), resident in SBUF
            idx_sb = sb.tile([128, n_blk * wcols], i16)
            nc.sync.dma_start(idx_sb[:], idxw[:])

            dinv = sb.tile([128, n_blk], fp)  # per-block node-major columns
            dinv128 = sb.tile([128, nsh], fp)  # broadcast across partitions
            h1T = sb.tile([128, nsh], fp)  # rows 0..h1-1 used
            muLT = sb.tile([128, nsh], fp)  # rows 0..h23-1 = [muT; logvarT]
            cc1_sb = sb.tile([128, ST * TROW], f16)
            cc2_sb = sb.tile([128, ST * TROW], f16)
            tbl = sb.tile([128, N_CORES * ST * TROW], f16)
            # zT split into 4 column-group tiles (all at partition base 0)
            cpg = n_nodes // 4  # columns per group
            ztg = [sb.tile([h2, cpg], fp, name=f"ztg{q}") for q in range(4)]

            nc.vector.memset(cc1_sb[:], 0.0)
            nc.vector.memset(cc2_sb[:], 0.0)

            # internal DRAM
            cc1_in = dr.tile([128, ST * TROW], f16)
            t1 = dr.tile([N_CORES * 128, ST * TROW], f16, addr_space="Shared")
            cc2_in = dr.tile([128, ST * TROW], f16)
            t2 = dr.tile([N_CORES * 128, ST * TROW], f16, addr_space="Shared")
            cc3_in = dr.tile([h2, nsh], fp)
            ztd = dr.tile([h2 * N_CORES, nsh], fp, addr_space="Shared")

            # ---------------- phase A: xw = x @ W1, deg, dinv, table 1 -------
            for b in range(n_blk):
                r0 = b * 128
                xb = wk.tile([128, f_in], fp, name="xb")
                nc.sync.dma_start(xb[:], xs[r0 : r0 + 128, :])
                xt4 = wk.tile([128, 4 * 128], fp, name="xt4")
                for k in range(f_in // 128):
                    tp = ps.tile([128, 128], fp, name="tp", tag="tp")
                    nc.tensor.transpose(
                        tp[:], xb[:, k * 128 : (k + 1) * 128], ident[:]
                    )
                    nc.vector.tensor_copy(xt4[:, k * 128 : (k + 1) * 128], tp[:])
                xwp = ps.tile([128, h1], fp, name="xwp", tag="acc")
                for k in range(f_in // 128):
                    nc.tensor.matmul(
                        xwp[:],
                        lhsT=xt4[:, k * 128 : (k + 1) * 128],
                        rhs=w1sb[:, k * h1 : (k + 1) * h1],
                        start=(k == 0),
                        stop=(k == f_in // 128 - 1),
                    )

                # degree from pad counts: deg = s_slots - #(idx == n_nodes)
                ixp = wk.tile([128, s_slots], fp, name="ixp")
                nc.sync.dma_start(ixp[:], idxp[r0 : r0 + 128, :])
                eq = wk.tile([128, s_slots], fp, name="eq")
                nc.vector.tensor_scalar(
                    eq[:], ixp[:], float(n_nodes), None, op0=OP.is_equal
                )
                cnt = wk.tile([128, 1], fp, name="cnt")
                nc.vector.reduce_sum(cnt[:], eq[:], axis=mybir.AxisListType.X)
                degt = wk.tile([128, 1], fp, name="degt")
                nc.vector.tensor_scalar(
                    degt[:], cnt[:], -1.0, float(s_slots), op0=OP.mult, op1=OP.add
                )
                rec = wk.tile([128, 1], fp, name="rec")
                nc.vector.reciprocal(rec[:], degt[:])
                nc.scalar.sqrt(dinv[:, b : b + 1], rec[:])

                # table row: dinv-scaled xw, fp16, features in cols 0..h1
                nc.vector.tensor_scalar(
                    cc1_sb[:, b * TROW : b * TROW + h1],
                    xwp[:],
                    dinv[:, b : b + 1],
                    None,
                    op0=OP.mult,
                )

            # dinv broadcast across partitions: per block transpose the
            # free-broadcast column (out[j, p] = dinv[p, b] for all j)
            for b in range(n_blk):
                dbp = ps.tile([128, 128], fp, name="dbp", tag="tp")
                nc.tensor.transpose(
                    dbp[:], dinv[:, b : b + 1].to_broadcast([128, 128]), ident[:]
                )
                nc.vector.tensor_copy(dinv128[:, b * 128 : (b + 1) * 128], dbp[:])

            nc.sync.dma_start(cc1_in[:], cc1_sb[:])
            nc.gpsimd.collective_compute(
                "AllGather",
                mybir.AluOpType.bypass,
                replica_groups=rg,
                ins=[cc1_in[:]],
                outs=[t1[:]],
            )
            # load gathered table into SBUF: partition p <- t1 row r*128+p
            nc.sync.dma_start(
                tbl[:].rearrange("p (r e) -> p r e", r=N_CORES),
                t1[:].rearrange("(r p) e -> p r e", p=128),
            )

            def gather_reduce(b):
                """Gather this block's slots from `tbl`, slot-sum -> [128,128] f16."""
                g1 = wk.tile([128, 128 * s_slots], f16, name="g1")
                ncall = (128 * s_slots + GIDX - 1) // GIDX
                for gci in range(ncall):
                    i0 = gci * GIDX
                    i1 = min(i0 + GIDX, 128 * s_slots)
                    nc.gpsimd.dma_gather(
                        out_ap=g1[:, i0:i1].rearrange("p (a c) -> p a c", a=1),
                        in_ap=tbl[:],
                        idxs_ap=idx_sb[
                            :, b * wcols + i0 // 16 : b * wcols + i1 // 16
                        ],
                        num_idxs=i1 - i0,
                        num_idxs_reg=i1 - i0,
                        elem_size=TROW,
                        transpose=True,
                        sbuf_tokens_per_rank=128,
                        sbuf_free_dim_per_rank=2 * TROW,
                        single_packet=False,
                    )
                red = wk.tile([128, 128], fp, name="red")
                nc.vector.reduce_sum(
                    red[:],
                    g1[:].rearrange("p (n s) -> p n s", s=s_slots),
                    axis=mybir.AxisListType.X,
                )
                return red

            # ---------------- phase B: h1T, table 2 ----------------
            import os
            B_PARTS = int(os.environ.get("B_PARTS", "4"))
            for b in range(n_blk if phases >= 2 else 0):
                r0 = b * 128
                red = gather_reduce(b)
                if B_PARTS < 2:
                    nc.vector.tensor_copy(h1T[:, r0 : r0 + 128], red[:])
                    continue
                # h1T = red * dinv(node, free) + b1(feature, partition)
                nc.vector.tensor_tensor(
                    h1T[:, r0 : r0 + 128],
                    red[:],
                    dinv128[:, r0 : r0 + 128],
                    op=OP.mult,
                )
                nc.vector.tensor_scalar(
                    h1T[:, r0 : r0 + 128],
                    h1T[:, r0 : r0 + 128],
                    b1sb[:, :1],
                    None,
                    op0=OP.add,
                )
                if B_PARTS < 3:
                    continue
                # hw23 = (h1 @ W23) node-major, then dinv-scale into table 2
                hwp = ps.tile([128, h23], fp, name="hwp", tag="acc")
                nc.tensor.matmul(
                    hwp[:],
                    lhsT=h1T[:h1, r0 : r0 + 128],
                    rhs=w23sb[:],
                    start=True,
                    stop=True,
                )
                nc.vector.tensor_scalar(
                    cc2_sb[:, b * TROW : b * TROW + h23],
                    hwp[:],
                    dinv[:, b : b + 1],
                    None,
                    op0=OP.mult,
                )

            if phases >= 2:
                nc.sync.dma_start(cc2_in[:], cc2_sb[:])
                nc.gpsimd.collective_compute(
                    "AllGather",
                    mybir.AluOpType.bypass,
                    replica_groups=rg,
                    ins=[cc2_in[:]],
                    outs=[t2[:]],
                )
                nc.sync.dma_start(
                    tbl[:].rearrange("p (r e) -> p r e", r=N_CORES),
                    t2[:].rearrange("(r p) e -> p r e", p=128),
                )

            # ---------------- phase C: muT / logvarT -----------------------
            for b in range(n_blk if phases >= 3 else 0):
                r0 = b * 128
                red = gather_reduce(b)
                nc.vector.tensor_tensor(
                    muLT[:, r0 : r0 + 128],
                    red[:],
                    dinv128[:, r0 : r0 + 128],
                    op=OP.mult,
                )
                nc.vector.tensor_scalar(
                    muLT[:, r0 : r0 + 128],
                    muLT[:, r0 : r0 + 128],
                    b23sb[:, :1],
                    None,
                    op0=OP.add,
                )
                # node-major [128, h23] for mu / logvar outputs
                mtp = ps.tile([128, h23], fp, name="mtp", tag="tp")
                nc.tensor.transpose(
                    mtp[:], muLT[:h23, r0 : r0 + 128], ident[:h23, :h23]
                )
                ml = wk.tile([128, h23], fp, name="ml")
                nc.vector.tensor_copy(ml[:], mtp[:])
                nc.sync.dma_start(muo[r0 : r0 + 128, :], ml[:, :h2])
                nc.sync.dma_start(lvo[r0 : r0 + 128, :], ml[:, h2:])

            if phases >= 3:
                nc.sync.dma_start(cc3_in[:], muLT[:h2, :])
                nc.gpsimd.collective_compute(
                    "AllGather",
                    mybir.AluOpType.bypass,
                    replica_groups=rg,
                    ins=[cc3_in[:]],
                    outs=[ztd[:]],
                )
                # rearrange rank stripes into the 4 column-group tiles
                for r in range(N_CORES):
                    q, s = r // 2, r % 2
                    nc.sync.dma_start(
                        ztg[q][:, s * nsh : (s + 1) * nsh],
                        ztd[h2 * r : h2 * (r + 1), :],
                    )

            # ---------------- phase D: adj = sigmoid(z z^T) -----------------
            CW = 2048 if n_nodes % 2048 == 0 else 512  # columns per output DMA
            cpg512 = cpg // 512  # 512-col chunks per group
            for rb in range(n_blk if phases >= 4 else 0):
                r0 = rb * 128
                for cg in range(n_nodes // CW):
                    adjsb = wk.tile([128, CW], fp, name="adjsb")
                    for cc in range(CW // 512):
                        c = cg * (CW // 512) + cc
                        q = c // cpg512
                        lc = c % cpg512
                        pa = ps.tile([128, 512], fp, name="pa")
                        nc.tensor.matmul(
                            pa[:],
                            lhsT=muLT[:h2, r0 : r0 + 128],
                            rhs=ztg[q][:, lc * 512 : (lc + 1) * 512],
                            start=True,
                            stop=True,
                        )
                        nc.scalar.activation(
                            adjsb[:, cc * 512 : (cc + 1) * 512], pa[:], AF.Sigmoid
                        )
                    nc.sync.dma_start(
                        adj[r0 : r0 + 128, cg * CW : (cg + 1) * CW], adjsb[:]
                    )

    nc.compile()
    return nc


# ---------------------------------------------------------------------------
# Host-side sharding / index construction
# ---------------------------------------------------------------------------


def build_inputs(x, edge_index, W1, b1, W2, b2, W3, b3, n_nodes, s_slots=None):
    """Build per-core input maps.  Returns (in_maps, s_slots)."""
    N = n_nodes
    nsh = N // N_CORES
    n_blk = nsh // 128

    src = np.concatenate(
        [np.asarray(edge_index[0]), np.arange(N, dtype=np.int64)]
    ).astype(np.int64)
    dst = np.concatenate(
        [np.asarray(edge_index[1]), np.arange(N, dtype=np.int64)]
    ).astype(np.int64)
    deg = np.bincount(dst, minlength=N)
    if s_slots is None:
        s_slots = max(64, int(deg.max()))
    assert deg.max() <= s_slots

    order = np.argsort(dst, kind="stable")
    sdst = dst[order]
    ssrc = src[order]
    offs = np.zeros(N + 1, np.int64)
    np.cumsum(deg, out=offs[1:])
    slot = np.arange(len(sdst), dtype=np.int64) - offs[sdst]
    tbl = np.full((N, s_slots), N, np.int64)
    tbl[sdst, slot] = ssrc

    wcols = (128 * s_slots) // 16
    x = np.ascontiguousarray(np.asarray(x, np.float32))
    w1 = np.ascontiguousarray(np.asarray(W1, np.float32))
    w23 = np.ascontiguousarray(
        np.concatenate([np.asarray(W2), np.asarray(W3)], axis=1).astype(np.float32)
    )
    h2 = np.asarray(W2).shape[1]
    b1cv = np.zeros((128, 1), np.float32)
    b1cv[: len(np.asarray(b1)), 0] = np.asarray(b1, np.float32)
    b23cv = np.zeros((128, 1), np.float32)
    b23cv[:h2, 0] = np.asarray(b2, np.float32)
    b23cv[h2 : 2 * h2, 0] = np.asarray(b3, np.float32)

    # token id for node g: rank r = g//nsh, stripe st = (g%nsh)//128,
    # partition p = g%128  ->  (r*(n_blk+1) + st)*128 + p.
    # pad (value N) -> rank 0's zero stripe, token n_blk*128.
    ST = n_blk + 1
    r_ = tbl // nsh
    loc = tbl % nsh
    tok = (r_ * ST + loc // 128) * 128 + (tbl % 128)
    tok = np.where(tbl == N, n_blk * 128, tok)

    in_maps = []
    for r in range(N_CORES):
        lo = r * nsh
        tok_r = tok[lo : lo + nsh]
        idxw = np.zeros((128, n_blk * wcols), np.int16)
        for b in range(n_blk):
            # node-major stream: i = n_local*s_slots + s
            unw = tok_r[b * 128 : (b + 1) * 128].reshape(-1)
            blk = unw.reshape(wcols, 16).T.astype(np.int16)
            for kq in range(8):  # replicated per gpsimd core (16-part groups)
                idxw[16 * kq : 16 * (kq + 1), b * wcols : (b + 1) * wcols] = blk
        in_maps.append(
            {
                "xs": np.ascontiguousarray(x[lo : lo + nsh]),
                "w1": w1,
                "w23": w23,
                "b1c": b1cv,
                "b23c": b23cv,
                "idxw": idxw,
                "idxp": tbl[lo : lo + nsh].astype(np.float32),
            }
        )
    return in_maps, s_slots


# ---------------------------------------------------------------------------
# Public entry point
# ---------------------------------------------------------------------------

_CACHE = {}


def run(x, edge_index, W1, b1, W2, b2, W3, b3, num_nodes, trace=False, **kw):
    from concourse.bass_utils import run_bass_kernel_spmd

    N = int(num_nodes)
    f_in = x.shape[1]
    h1 = W1.shape[1]
    h2 = W2.shape[1]
    nsh = N // N_CORES

    in_maps, s_slots = build_inputs(
        x, edge_index, W1, b1, W2, b2, W3, b3, N
    )
    key = (N, nsh, s_slots, f_in, h1, h2)
    if key not in _CACHE:
        _CACHE[key] = build_kernel(N, nsh, s_slots, f_in, h1, h2)
    nc = _CACHE[key]

    res = run_bass_kernel_spmd(
        nc, in_maps, core_ids=list(range(N_CORES)), trace=trace, **kw
    )
    outs = res.results
    adj = np.concatenate([o["adj"] for o in outs], axis=0)
    mu = np.concatenate([o["muo"] for o in outs], axis=0)
    lv = np.concatenate([o["lvo"] for o in outs], axis=0)
    return (adj, mu, lv), res


def kernel(x, edge_index, W1, b1, W2, b2, W3, b3, num_nodes):
    (adj, mu, lv), _ = run(x, edge_index, W1, b1, W2, b2, W3, b3, num_nodes)
    return adj, mu, lv
